# revision 16
# baseline (speedup 1.0000x reference)
"""CKA (centered kernel alignment) on 8 Trainium2 NeuronCores.

Math: with H = I - 11^T/n, H G H = (Hz)(Hz)^T, so each HSIC term is the
Frobenius norm^2 of a feature-covariance block of C = zc^T zc where
zc = [x - colmean(x) | y - colmean(y)] (8192 x 3072):
    hsic_xy = ||C[x-cols, y-cols]||_F^2   (etc.)
Column-centering happens on the HOST (exact), so the device only computes
C's upper-triangle 128x128 blocks and partial sums of squares -- no
centering pass, no column sums, and (crucially) NO collectives.

Sharding: the 24 column-tiles (128 wide) of zc form 300 unordered tile
pairs {a,b} (24 diagonal + 276 off-diagonal).  Pairs are covered by a
rotation design: core r computes blocks {(s+3r)%24, (s+3r+d)%24} for
s in {0,1,2}, d in 0..12 (312 block instances; the 12 d=12 pairs are
computed twice, weighted 1/2 on the host).  Each core therefore needs
only 15 consecutive (mod 24) column tiles -- the host pre-rotates and
packs them, so the device program is rank-uniform: fixed SBUF offsets,
different data.

Inputs are quantized to fp8e4 on the host (validated: rel-err ~3e-4 on
the final scalar vs the f64 reference; tolerance is 2e-2).  fp8 makes
the per-core panel 15.7 MB so it is fully SBUF-resident, and enables
DoubleRow matmuls (2 contraction rows per PE cell) for ~1.4x PE rate.

Device program per core:
  - 32 pair DMAs (2 ktiles each, matching DoubleRow granularity) plus a
    tiny first DMA feeding a dedicated buffer so the first matmul can
    fire as soon as its data lands
  - 8 HAM warm-up matmuls on a zeroed tile bridge the DMA-semaphore
    latency so the real stream runs at 2.4 GHz from its first op
  - phase A (s=0,1): 8 PSUM chains, t-outer so matmuls start as each
    pair lands (DMA/compute overlap); phases B/C/D (s=2) on resident
    data, ordered so the final phase is the single-subcol w1 chain
  - per chain end: square+reduce per 128-col sub-block into
    acc[128, 48], alternating ACT (fused Square+accum) and DVE
    (copy/mul/reduce) so consecutive squares run on two engines;
    host sums partitions and applies pair weights + CKA formula in f64.
"""

import os

import numpy as np

N = 8192               # examples
NT = 24                # 128-col tiles of z = [x | y] (2048 + 1024 = 3072)
NXT = 16               # tiles belonging to x
RES = 15               # resident tiles per core (positions 0..14)
DW = RES * 128         # 1920
P = 128
KT = N // P            # 64 contraction tiles
N_CORES = 8
RUNW = (4, 4, 4, 1)    # partner-run widths per sigma (partners d = 0..12)

_DR = os.environ.get("CKA_DR", "1") == "1"   # DoubleRow fp8 matmuls

_COMPILED = None


def _build():
    import concourse.bacc as bacc
    import concourse.mybir as mybir
    import concourse.tile as tile

    f32 = mybir.dt.float32
    f8 = mybir.dt.float8e4

    nc = bacc.Bacc("TRN2", target_bir_lowering=False, debug=False,
                   num_devices=N_CORES)
    z = nc.dram_tensor("z", [N, DW], f8, kind="ExternalInput")
    out = nc.dram_tensor("partials", [P, 48], f32, kind="ExternalOutput")

    with tile.TileContext(nc) as tc:
        with (
            tc.tile_pool(name="persist", bufs=1) as persist,
            tc.tile_pool(name="spill", bufs=2) as spill,
            tc.tile_pool(name="psum", bufs=8, space="PSUM") as psum,
        ):
            zb = persist.tile([P, KT, DW], f8)
            zb0 = persist.tile([P, 2, 512], f8)
            if _DR:
                # Dedicated small buffer for the very first matmul (sigma=0,
                # j=0, t=0: lhs col 0, rhs cols 0:512) -- a single tiny DMA
                # dependency, so the PE starts as soon as possible.
                nc.sync.dma_start(
                    zb0[:, :, :],
                    z[0:2 * P, 0:512].rearrange("(h p) w -> p h w", p=P))
                # 2-ktile batches: one DMA per DoubleRow contraction pair
                for t in range(KT // 2):
                    nc.sync.dma_start(
                        zb[:, 2 * t:2 * t + 2, :],
                        z[2 * t * P:(2 * t + 2) * P, :].rearrange(
                            "(h p) w -> p h w", p=P))
            else:
                for k in range(KT):
                    nc.sync.dma_start(zb[:, k, :], z[k * P:(k + 1) * P, :])

            acc = persist.tile([P, 48], f32)
            nc.vector.memset(acc[:], 0.0)

            if _DR:
                # HAM warm-up: dummy matmuls on a zeroed tile keep the PE
                # busy while the first DMA lands, so the clock gate opens
                # (1.2 -> 2.4 GHz) before the real stream begins.
                zw = persist.tile([P, 2, 512], f8)
                nc.vector.memset(zw[:], 0.0)
                psw = psum.tile([P, 512], f32, tag="ps", name="ps")
                for _ in range(8):
                    nc.tensor.matmul(
                        psw[:], zw[:, :, 0:128], zw[:, :, :],
                        start=True, stop=True,
                        perf_mode=mybir.MatmulPerfMode.DoubleRow)
                junkw = spill.tile([P, P], f32, tag="junk", name="junk",
                                   bufs=4)
                nc.scalar.copy(junkw[:], psw[:, 0:P])

            def phase(slots):
                pss = {}
                for s, j in slots:
                    pss[s, j] = psum.tile([P, 512], f32, tag="ps", name="ps")
                if _DR:
                    for t in range(KT // 2):
                        for s, j in slots:
                            w = RUNW[j] * P
                            c0 = (s + 4 * j) * P
                            if t == 0 and (s, j) == (0, 0):
                                lhs = zb0[:, :, 0:P]
                                rhs = zb0[:, :, :]
                            else:
                                lhs = zb[:, 2 * t:2 * t + 2,
                                         s * P:(s + 1) * P]
                                rhs = zb[:, 2 * t:2 * t + 2, c0:c0 + w]
                            nc.tensor.matmul(
                                pss[s, j][:, 0:w], lhs, rhs,
                                start=(t == 0), stop=(t == KT // 2 - 1),
                                perf_mode=mybir.MatmulPerfMode.DoubleRow)
                else:
                    for k in range(KT):
                        for s, j in slots:
                            lhs = zb[:, k, s * P:(s + 1) * P]
                            w = RUNW[j] * P
                            c0 = (s + 4 * j) * P
                            nc.tensor.matmul(
                                pss[s, j][:, 0:w], lhs,
                                zb[:, k, c0:c0 + w],
                                start=(k == 0), stop=(k == KT - 1))
                # square + reduce each 128-col sub-block into its acc column,
                # alternating ACT (fused square+accum) and DVE (mul+reduce)
                # so consecutive squares run on two engines in parallel.
                n_sq = 0
                for s, j in slots:
                    for t4 in range(RUNW[j]):
                        ps = pss[s, j][:, t4 * P:(t4 + 1) * P]
                        junk = spill.tile([P, P], f32, tag="junk",
                                          name="junk", bufs=4)
                        col = s * 16 + j * 4 + t4
                        if n_sq % 2 == 0:
                            nc.scalar.activation(
                                junk[:], ps,
                                mybir.ActivationFunctionType.Square,
                                accum_out=acc[:, col:col + 1])
                        else:
                            nc.vector.tensor_copy(junk[:], ps)
                            sq = spill.tile([P, P], f32, tag="junk",
                                            name="junk", bufs=4)
                            nc.vector.tensor_mul(sq[:], junk[:], junk[:])
                            nc.vector.tensor_reduce(
                                out=acc[:, col:col + 1], in_=sq[:],
                                axis=mybir.AxisListType.X,
                                op=mybir.AluOpType.add)
                        n_sq += 1

            phase([(s, j) for s in (0, 1) for j in range(4)])
            nc.sync.dma_start(out[:, 0:32], acc[:, 0:32])
            phase([(2, 0), (2, 1)])
            phase([(2, 2)])
            phase([(2, 3)])
            nc.sync.dma_start(out[:, 32:48], acc[:, 32:48])

    nc.compile()
    return nc


def _get_compiled():
    global _COMPILED
    if _COMPILED is None:
        _COMPILED = _build()
    return _COMPILED


def _pack_inputs(x, y):
    """Center columns, quantize to fp8e4, build each core's rotated panel."""
    import ml_dtypes
    x = np.asarray(x)
    y = np.asarray(y)
    xc = (x - x.mean(axis=0, dtype=np.float64).astype(np.float32))
    yc = (y - y.mean(axis=0, dtype=np.float64).astype(np.float32))
    xq = xc.astype(ml_dtypes.float8_e4m3)
    yq = yc.astype(ml_dtypes.float8_e4m3)
    tiles = ([xq[:, c * P:(c + 1) * P] for c in range(NXT)]
             + [yq[:, c * P:(c + 1) * P] for c in range(NT - NXT)])
    in_maps = []
    for r in range(N_CORES):
        cols = [(3 * r + p) % NT for p in range(RES)]
        zr = np.ascontiguousarray(
            np.concatenate([tiles[c] for c in cols], axis=1))
        in_maps.append({"z": zr})
    return in_maps


def _combine(partials):
    """Host reduction: weighted sums of per-block ssq -> CKA scalar."""
    hxx = hxy = hyy = 0.0
    for r in range(N_CORES):
        p = np.asarray(partials[r], dtype=np.float64)
        colsums = p.sum(axis=0)
        for s in range(3):
            a = (3 * r + s) % NT
            for j in range(4):
                for t4 in range(RUNW[j]):
                    d = 4 * j + t4
                    b = (3 * r + s + d) % NT
                    ssq = colsums[s * 16 + j * 4 + t4]
                    cov = 2.0 if d == 12 else 1.0
                    ax, bx = a < NXT, b < NXT
                    if ax and bx:
                        hxx += (1.0 if d == 0 else 2.0) / cov * ssq
                    elif not ax and not bx:
                        hyy += (1.0 if d == 0 else 2.0) / cov * ssq
                    else:
                        hxy += 1.0 / cov * ssq
    return np.float32(hxy / (np.sqrt(hxx * hyy) + 1e-8))


def _run(x, y, trace=False):
    import time
    from concourse import bass_utils
    nc = _get_compiled()
    in_maps = _pack_inputs(x, y)
    last_err = None
    for attempt in range(3):
        try:
            res = bass_utils.run_bass_kernel_spmd(
                nc, in_maps, core_ids=list(range(N_CORES)), trace=trace)
            break
        except Exception as e:  # transient device wedge: retry
            last_err = e
            time.sleep(5.0)
    else:
        raise last_err
    val = _combine([res.results[r]["partials"] for r in range(N_CORES)])
    return np.asarray(val, dtype=np.float32), res


def kernel(x, y):
    val, _ = _run(x, y, trace=False)
    return val
